# revision 54
# baseline (speedup 1.0000x reference)
"""Causal multi-head self-attention with RoPE on 8 TRN2 NeuronCores.

Problem: B=2, S=2048, D=2048, H=16 heads (dk=128), causal, interleaved RoPE.

Sharding (hardcoded): core c handles batch b = c // 4 and head group
g = c % 4 (heads 4g..4g+3, a 512-wide slice of d_model).  Attention is
embarrassingly parallel over (batch, head-group); the output projection is
row-parallel (each core contracts its 512-slice of attnout against the
matching 512 columns of Wo), so each core returns a full-size partial
output and the host sums the 4 partials per batch.

All device matmuls run in fp16 (full TensorE rate) with fp32 PSUM
accumulation.  Layout is fully transposed ("T" layout, feature dim on
partitions) so no on-device transposes are needed anywhere:

  xT [d, s] --(W.T @ .)--> QT/KT [dk, s] --RoPE--> scores.T [k, q]
  --exp--> P.T [k, q] --(V natural-layout matmul)--> OT [dv, q] --Wo--> outT

v2 scheduling: the softmax denominator no longer burns TensorE cycles as a
ones-stationary matmul per key block.  P tiles are accumulated on VectorE
(fp16) and a single ones-matmul per (head, q-chunk) does the final
partition reduction.  Because the exp on ScalarE ((n+352)/1.2 ns) is slower
than the remaining per-step PE work (QK+AV ~ n/1.2 ns), attention steps are
woven at fine grain with independent projection / output-projection matmuls
("filler") so the PE queue never head-of-line blocks on ScalarE.  Attention
for q-chunk sc runs inside iteration sc (right after its Q projection), so
no Scalar-paced attention epilogue remains; the kernel ends on the last
output-projection chain.  Input DMAs are issued from both the Sync and
GpSimd queues to halve the serial ~0.65us-per-descriptor issue cost at the
head of the kernel.
"""

import numpy as np

import concourse.bass as bass
import concourse.mybir as mybir
import concourse.tile as tile
from concourse import bacc
from concourse import bass_utils

B = 2
S = 2048
D = 2048
H = 16
DK = 128
HPC = 4          # heads per core
G = HPC * DK     # 512, d_model slice per core
NC = 8
THETA = 10000.0
SCALE = 1.0 / DK ** 0.5
EXP_BIAS = -5.0  # exp(s*SCALE - 5): keeps fp16 P in range; cancels in norm

FP16 = mybir.dt.float16
FP32 = mybir.dt.float32

_BUILT = None  # cached compiled Bass module


def _build_kernel(tc, out_d, xT_d, wqT_d, wkT_d, wvT_d, woT_d, ropeC_d,
                  ropeS_d, masks_d, ones_d):
    nc = tc.nc
    NSC = S // 512          # 4 s-chunks
    NDC = D // 128          # 16 d-chunks (contraction)
    shuffle_mask = [i + 1 if i % 2 == 0 else i - 1 for i in range(32)]

    with (
        tc.tile_pool(name="consts", bufs=1) as consts,
        tc.tile_pool(name="wqkv", bufs=1) as wqkv,
        tc.tile_pool(name="xin", bufs=2) as xin,
        tc.tile_pool(name="persist", bufs=1) as persist,
        tc.tile_pool(name="ropetmp", bufs=2) as ropetmp,
        tc.tile_pool(name="ptile", bufs=1) as ptile,
        tc.tile_pool(name="accp", bufs=4) as accp,
        tc.tile_pool(name="stage", bufs=2) as stage,
        tc.tile_pool(name="mm", bufs=3, space="PSUM") as mmp,
        tc.tile_pool(name="psST", bufs=2, space="PSUM") as psST,
        tc.tile_pool(name="psOT", bufs=2, space="PSUM") as psOT,
        tc.tile_pool(name="psZ", bufs=1, space="PSUM") as psZ,
    ):
        # weights in SBUF as [128, dc*512 + o]
        wq = wqkv.tile([128, NDC * G], FP16, tag="wq")
        wk = wqkv.tile([128, NDC * G], FP16, tag="wk")
        wv = wqkv.tile([128, NDC * G], FP16, tag="wv")
        wo = wqkv.tile([128, HPC * D], FP16, tag="wo")   # [128, hc*2048 + o]
        # persistent activations
        qrot = persist.tile([128, HPC * S], FP16, tag="qrot")  # [dk, h*S+s]
        krot = persist.tile([128, HPC * S], FP16, tag="krot")
        vN = persist.tile([128, (S // 128) * G], FP16, tag="vN")  # [s%, sb*G+dv]
        oT = persist.tile([128, HPC * S], FP16, tag="oT")      # [dv, h*S+q]

        ropeC = ropeS = maskT = onesT = expbias = None

        # PE warm-up: a short burst of dummy matmuls opens the HAM clock
        # gate (1.2 -> 2.4 GHz) while the first x/wq DMA pieces land; real
        # matmuls take over DMA-paced right after, keeping the PE active.
        warm = consts.tile([128, 512], FP16, tag="warm")
        nc.vector.memset(warm[:], 0.0)
        wps = psZ.tile([128, 512], FP32, tag="psZ", name="warmps")
        for _ in range(5):
            nc.tensor.matmul(wps[:], lhsT=warm[:, :128], rhs=warm[:],
                             start=True, stop=True)

        # ---- filler machinery: one closure == one PE matmul (plus any
        # cheap same-phase tail ops on other engines).  Attention steps
        # pump the cursor so the PE queue always holds independent work
        # while ScalarE chews on the exps.
        filler = []
        cursor = [0]
        credit = [0.0]

        def pump_upto(idx):
            while cursor[0] < min(idx, len(filler)):
                filler[cursor[0]]()
                cursor[0] += 1

        def pump(rate):
            credit[0] += rate
            n = int(credit[0])
            credit[0] -= n
            pump_upto(cursor[0] + n)

        def pump_all():
            pump_upto(len(filler))
            credit[0] = 0.0

        def emit_rope(dst, sc, h, ps):
            """RoPE from the PSUM projection result into dst[dk, h*S+sc*512]."""
            raw = ropetmp.tile([128, 512], FP16, tag="raw")
            nc.vector.tensor_copy(raw[:], ps[:])
            swp = ropetmp.tile([128, 512], FP16, tag="swp")
            nc.vector.stream_shuffle(swp[:], raw[:], shuffle_mask)
            t1 = ropetmp.tile([128, 512], FP16, tag="t1")
            csl = slice(sc * 512, (sc + 1) * 512)
            nc.vector.tensor_mul(t1[:], raw[:], ropeC[:, csl])
            t2 = ropetmp.tile([128, 512], FP16, tag="t2")
            nc.vector.tensor_mul(t2[:], swp[:], ropeS[:, csl])
            dsl = slice(h * S + sc * 512, h * S + (sc + 1) * 512)
            nc.vector.tensor_add(dst[:, dsl], t1[:], t2[:])

        def chain_qk(w_s, dst, xsc, sc, h, emit):
            """One head's QT/KT projection chain for chunk sc + fused RoPE."""
            ps = mmp.tile([128, 512], FP32, tag="mm", name="qkg")
            for dc in range(NDC):
                def mmfn(dc=dc, ps=ps):
                    nc.tensor.matmul(
                        ps[:],
                        lhsT=w_s[:, dc * G + h * 128: dc * G + (h + 1) * 128],
                        rhs=xsc[:, dc * 512:(dc + 1) * 512],
                        start=(dc == 0), stop=(dc == NDC - 1),
                        skip_group_check=True,
                    )
                    if dc == NDC - 1:
                        emit_rope(dst, sc, h, ps)
                emit(mmfn)

        def chains_qk_dcouter(w_s, dst, xsc, sc, heads=(0, 1, 2, 3)):
            """Heads' QT/KT chains with the dc loop OUTER, so each arriving
            2-dc DMA piece enables len(heads) matmuls per dc instead of 1.
            Only worth it in iteration 0 where the chains are DMA-paced;
            extra PSUM accumulators beyond 2 are borrowed from the (still
            idle) attention score pool."""
            pools = [mmp, mmp, psST, psST]
            tags = ["mm", "mm", "psST", "psST"]
            pss = [pools[j].tile([128, 512], FP32, tag=tags[j], name="qkg")
                   for j in range(len(heads))]
            for dc in range(NDC):
                for j, h in enumerate(heads):
                    nc.tensor.matmul(
                        pss[j][:],
                        lhsT=w_s[:, dc * G + h * 128: dc * G + (h + 1) * 128],
                        rhs=xsc[:, dc * 512:(dc + 1) * 512],
                        start=(dc == 0), stop=(dc == NDC - 1),
                        skip_group_check=True,
                    )
            for j, h in enumerate(heads):
                emit_rope(dst, sc, h, pss[j])

        def chain_v(xsc, sc, sb, emit):
            """V projection (natural layout) for row-block sb of chunk sc."""
            ps = mmp.tile([128, 512], FP32, tag="mm", name="vg")
            for dc in range(NDC):
                def mmfn(dc=dc, ps=ps):
                    nc.tensor.matmul(
                        ps[:],
                        lhsT=xsc[:, dc * 512 + sb * 128:
                                 dc * 512 + (sb + 1) * 128],
                        rhs=wv[:, dc * G:(dc + 1) * G],
                        start=(dc == 0), stop=(dc == NDC - 1),
                        skip_group_check=True,
                    )
                    if dc == NDC - 1:
                        sblk = sc * 4 + sb
                        nc.scalar.copy(vN[:, sblk * G:(sblk + 1) * G], ps[:])
                emit(mmfn)

        def chain_op(sc, ob, k, emit, split_q=False, fine_dma=False):
            """Output-projection chain: columns ob*128.. for s-chunk sc.
            split_q: issue half the output DMAs from the Scalar queue so the
            final drain isn't serialized on Sync (epilogue only).  fine_dma:
            additionally split this chain's output into two half-descriptors
            (for the last chains before kernel end)."""
            ps = mmp.tile([128, 512], FP32, tag="mm", name="psD")
            for hc in range(HPC):
                def mmfn(hc=hc, ps=ps):
                    nc.tensor.matmul(
                        ps[:],
                        lhsT=wo[:, hc * D + ob * 128: hc * D + (ob + 1) * 128],
                        rhs=oT[:, hc * S + sc * 512: hc * S + (sc + 1) * 512],
                        start=(hc == 0), stop=(hc == HPC - 1),
                        skip_group_check=True,
                    )
                    if hc == HPC - 1:
                        so = stage.tile([128, 512], FP16, tag="so", bufs=4)
                        if k % 2 == 0:
                            nc.vector.tensor_copy(so[:], ps[:])
                            dma_eng = nc.sync
                        else:
                            nc.scalar.copy(so[:], ps[:])
                            dma_eng = nc.scalar if split_q else nc.sync
                        orows = slice(ob * 128, (ob + 1) * 128)
                        if fine_dma:
                            # halves on two queues: per-descriptor DMA rate
                            # is ~1 engine's worth, so this halves the final
                            # output-drain latency after the last matmul.
                            for half, eng2 in ((0, nc.sync), (1, nc.scalar)):
                                csl = slice(sc * 512 + half * 256,
                                            sc * 512 + (half + 1) * 256)
                                eng2.dma_start(
                                    out=out_d[orows, csl],
                                    in_=so[:, half * 256:(half + 1) * 256],
                                )
                        else:
                            dma_eng.dma_start(
                                out=out_d[orows,
                                          sc * 512:(sc + 1) * 512],
                                in_=so[:],
                            )
                emit(mmfn)

        def attn_pair(hpair, qj, mark_k, mark_v, dummy_fill=False):
            """Two heads' attention for q-chunk qj, woven with filler MMs.

            Front (QK matmul -> exp -> mask) runs LA steps ahead of the
            dependent AV matmul + VectorE P-accumulation; ~1.6 filler MMs
            per step cover ScalarE's (n+352)/1.2 exp cost so the PE queue
            never waits on the exp.  Diagonal blocks skip their fully-
            masked query-column prefix."""
            LA = 2
            ots = [psOT.tile([128, 512], FP32, tag="psOT", name=f"ot{i}")
                   for i in range(2)]
            accs = [accp.tile([128, 512], FP16, tag="acc", name=f"acc{i}")
                    for i in range(2)]
            nk = 4 * qj + 4
            steps = [(ki, i, h) for ki in range(nk)
                     for i, h in enumerate(hpair)]
            pending = []

            def emit_front(idx):
                ki, i, h = steps[idx]
                r = ki - 4 * qj
                qoff = 128 * r if r > 0 else 0  # fully-masked prefix width
                n = 512 - qoff
                qs0 = h * S + qj * 512
                if ki >= 4 * qj:  # diagonal: krot(qj, h) must be emitted
                    pump_upto(mark_k[h])
                st = psST.tile([128, 512], FP32, tag="psST")
                nc.tensor.matmul(
                    st[:, :n],
                    lhsT=krot[:, h * S + ki * 128: h * S + (ki + 1) * 128],
                    rhs=qrot[:, qs0 + qoff: qs0 + 512],
                    start=True, stop=True, skip_group_check=True,
                )
                pt = ptile.tile([128, 512], FP16, tag="pt", bufs=8)
                nc.scalar.activation(
                    pt[:, :n], st[:, :n],
                    mybir.ActivationFunctionType.Exp,
                    bias=expbias[:], scale=SCALE,
                )
                pa = pt
                if r >= 0:  # diagonal: zero the upper triangle
                    pm = ptile.tile([128, 512], FP16, tag="pm", bufs=5)
                    nc.vector.tensor_mul(
                        pm[:, :n], pt[:, :n],
                        maskT[:, r * 512 + qoff:(r + 1) * 512])
                    pa = pm
                return (ki, i, h, qoff, n, pa)

            def emit_back(item):
                ki, i, h, qoff, n, pa = item
                r = ki - 4 * qj
                if r >= 0:  # diagonal: vN(qj, sb=r) must be emitted
                    pump_upto(mark_v[r])
                nc.tensor.matmul(
                    ots[i][:, qoff:512],
                    lhsT=vN[:, ki * G + h * 128: ki * G + (h + 1) * 128],
                    rhs=pa[:, :n],
                    start=(ki == 0), stop=(ki == nk - 1),
                    skip_group_check=True,
                )
                if ki == 0:
                    nc.vector.tensor_copy(accs[i][:], pa[:])
                else:
                    nc.vector.tensor_add(accs[i][:, qoff:512],
                                         accs[i][:, qoff:512], pa[:, :n])

            def dfill():
                # real filler exhausted: burn the would-be PE idle on a
                # dummy matmul so the HAM clock gate stays open through
                # the ScalarE-paced stretch (a >3.4us PE gap re-throttles
                # the PE to 1.2 GHz for the next iteration's start).
                if dummy_fill and cursor[0] >= len(filler):
                    nc.tensor.matmul(wps[:, :256], lhsT=warm[:, :128],
                                     rhs=warm[:, :256], start=True,
                                     stop=True, skip_group_check=True)

            for idx in range(len(steps)):
                pending.append(emit_front(idx))
                pump(0.8)
                dfill()
                if len(pending) > LA:
                    emit_back(pending.pop(0))
                    pump(0.8)
                    dfill()
            for item in pending:
                emit_back(item)
                pump(0.8)
                dfill()

            def finalize():
                # Deferred by the caller until independent matmuls are in
                # the PE queue: the zt matmuls gate on VectorE's P-sum
                # backlog, and emitting them inline head-of-line blocks
                # the queue at every iteration boundary.
                for i, h in enumerate(hpair):
                    zt = psZ.tile([128, 512], FP32, tag="psZ", name="zt")
                    nc.tensor.matmul(zt[:], lhsT=onesT[:], rhs=accs[i][:],
                                     start=True, stop=True,
                                     skip_group_check=True)
                    qsl = slice(h * S + qj * 512, h * S + (qj + 1) * 512)
                    rz = stage.tile([128, 512], FP32, tag="rz")
                    nc.vector.reciprocal_approx_fast(out=rz[:], in_=zt[:])
                    nc.vector.tensor_mul(oT[:, qsl], ots[i][:], rz[:])
            return finalize

        run_now = lambda fn: fn()
        add_filler = filler.append

        # Pipeline: iteration sc does Q(sc) projection up front, then weaves
        # attention for q-chunk sc with the K(sc)/V(sc) chains and the
        # previous chunk's output projection as PE filler.
        fin23_prev = None
        for sc in range(NSC):
            assert cursor[0] == len(filler)
            filler.clear()
            cursor[0] = 0

            if sc == 0:
                xsc = xin.tile([128, NDC * 512], FP16, tag="xsc")
                # 1-dc pieces alternating between the Sync and Scalar issue
                # queues: first piece lands ~1us after the engines start, and
                # each piece unblocks 4 dc-outer matmuls.
                for dc in range(NDC):
                    eng = nc.sync if dc % 2 == 0 else nc.scalar
                    eng.dma_start(
                        out=xsc[:, dc * 512:(dc + 1) * 512],
                        in_=xT_d[dc * 128:(dc + 1) * 128, 0:512],
                    )
                for dc in range(0, NDC, 2):
                    nc.gpsimd.dma_start(
                        out=wq[:, dc * G:(dc + 2) * G]
                            .rearrange("p (c o) -> p c o", c=2),
                        in_=wqT_d[dc * 128:(dc + 2) * 128, :]
                            .rearrange("(c p) o -> p c o", p=128),
                    )
                for dc in range(0, NDC, 2):
                    nc.gpsimd.dma_start(
                        out=wk[:, dc * G:(dc + 2) * G]
                            .rearrange("p (c o) -> p c o", c=2),
                        in_=wkT_d[dc * 128:(dc + 2) * 128, :]
                            .rearrange("(c p) o -> p c o", p=128),
                    )
                ropeC = consts.tile_from(ropeC_d)    # [128, 2048] fp16
                ropeS = consts.tile_from(ropeS_d)
                maskT = consts.tile_from(masks_d)    # [128, 4*512] fp16
                expbias = consts.tile([128, 1], FP32, tag="expbias")
                nc.vector.memset(expbias[:], EXP_BIAS)
            else:
                xsc = xnext

            # Q projection for this chunk.  Iteration 0 is DMA-paced, so it
            # runs dc-outer (each arriving piece enables 8 matmuls); later
            # iterations have x resident and run per-head (PE-bound).
            if sc == 0:
                chains_qk_dcouter(wq, qrot, xsc, sc)
            else:
                for h in range(HPC):
                    chain_qk(wq, qrot, xsc, sc, h, run_now)

            if sc == 0:
                for dc in range(0, NDC, 2):
                    nc.gpsimd.dma_start(
                        out=wv[:, dc * G:(dc + 2) * G]
                            .rearrange("p (c o) -> p c o", c=2),
                        in_=wvT_d[dc * 128:(dc + 2) * 128, :]
                            .rearrange("(c p) o -> p c o", p=128),
                    )
                onesT = consts.tile_from(ones_d)     # [128, 128] fp16
                nc.gpsimd.dma_start(
                    out=wo[:].rearrange("p (c o) -> p c o", c=HPC),
                    in_=woT_d.rearrange("(c p) o -> p c o", p=128),
                )

            if fin23_prev is not None:
                fin23_prev()
            fin23_prev = None

            # Filler for the weave: K chains, V chains, out-proj(sc-1).
            mark_k = {}
            mark_v = {}
            if sc == 0:
                chains_qk_dcouter(wk, krot, xsc, sc)
                for h in range(HPC):
                    mark_k[h] = 0
            else:
                for h in range(HPC):
                    chain_qk(wk, krot, xsc, sc, h, add_filler)
                    mark_k[h] = len(filler)
            for sb in range(4):
                chain_v(xsc, sc, sb, add_filler)
                mark_v[sb] = len(filler)
            if sc >= 1:
                for ob in range(16):
                    chain_op(sc - 1, ob, ob, add_filler)

            fin01 = attn_pair((0, 1), sc, mark_k, mark_v)
            # prefetch next chunk of x once iteration 0's critical input
            # DMAs have drained; it lands well before iteration sc+1.
            if sc + 1 < NSC:
                xnext = xin.tile([128, NDC * 512], FP16, tag="xsc")
                for dc in range(0, NDC, 2):
                    nc.sync.dma_start(
                        out=xnext[:, dc * 512:(dc + 2) * 512]
                            .rearrange("p (c s) -> p c s", c=2),
                        in_=xT_d[dc * 128:(dc + 2) * 128,
                                 (sc + 1) * 512:(sc + 2) * 512]
                            .rearrange("(c p) s -> p c s", p=128),
                    )
            fin23 = attn_pair((2, 3), sc, mark_k, mark_v,
                              dummy_fill=(sc == 0))
            fin01()
            pump_all()
            fin23_prev = fin23

        if fin23_prev is not None:
            fin23_prev()
        # epilogue: only the last chunk's output projection remains.  k=ob+1
        # so the final chain's stage copy lands on the faster Vector path.
        for ob in range(16):
            chain_op(NSC - 1, ob, ob + 1, run_now, split_q=True,
                     fine_dma=(ob >= 12))


def _get_built():
    global _BUILT
    if _BUILT is not None:
        return _BUILT
    nc = bacc.Bacc("TRN2", target_bir_lowering=False, debug=False,
                   enable_asserts=False, num_devices=NC)
    d = {}
    d["xT"] = nc.dram_tensor("xT", (D, S), FP16, kind="ExternalInput").ap()
    d["wqT"] = nc.dram_tensor("wqT", (D, G), FP16, kind="ExternalInput").ap()
    d["wkT"] = nc.dram_tensor("wkT", (D, G), FP16, kind="ExternalInput").ap()
    d["wvT"] = nc.dram_tensor("wvT", (D, G), FP16, kind="ExternalInput").ap()
    d["woT"] = nc.dram_tensor("woT", (G, D), FP16, kind="ExternalInput").ap()
    d["ropeC"] = nc.dram_tensor("ropeC", (DK, S), FP16,
                                kind="ExternalInput").ap()
    d["ropeS"] = nc.dram_tensor("ropeS", (DK, S), FP16,
                                kind="ExternalInput").ap()
    d["masks"] = nc.dram_tensor("masks", (DK, 4 * 512), FP16,
                                kind="ExternalInput").ap()
    d["ones"] = nc.dram_tensor("ones", (DK, DK), FP16,
                               kind="ExternalInput").ap()
    out_d = nc.dram_tensor("out", (D, S), FP16, kind="ExternalOutput").ap()
    with tile.TileContext(nc) as tc:
        _build_kernel(tc, out_d, d["xT"], d["wqT"], d["wkT"], d["wvT"],
                      d["woT"], d["ropeC"], d["ropeS"], d["masks"], d["ones"])
    nc.compile()
    _BUILT = nc
    return nc


def _host_tables(token_positions):
    pos = np.asarray(token_positions).astype(np.float64)       # [S]
    inv_freq = 1.0 / (THETA ** (np.arange(0, DK, 2, dtype=np.float64) / DK))
    ang = pos[None, :] * inv_freq[:, None]                     # [64, S]
    cos = np.cos(ang)
    sin = np.sin(ang)
    C = np.empty((DK, S), np.float16)
    Sm = np.empty((DK, S), np.float16)
    C[0::2] = cos
    C[1::2] = cos
    Sm[0::2] = -sin
    Sm[1::2] = sin
    # diagonal-block masks: mask_r[kr, qc] = 1 iff qc >= 128*r + kr
    masks = np.zeros((DK, 4 * 512), np.float16)
    kr = np.arange(128)[:, None]
    qc = np.arange(512)[None, :]
    for r in range(4):
        masks[:, r * 512:(r + 1) * 512] = (qc >= 128 * r + kr)
    ones = np.ones((DK, DK), np.float16)
    return C, Sm, masks, ones


def _make_in_maps(x, token_positions, Wq, Wk, Wv, Wo):
    C, Sm, masks, ones = _host_tables(token_positions)
    x = np.asarray(x, dtype=np.float32)
    Wq = np.asarray(Wq, dtype=np.float32)
    Wk = np.asarray(Wk, dtype=np.float32)
    Wv = np.asarray(Wv, dtype=np.float32)
    Wo = np.asarray(Wo, dtype=np.float32)
    xT = [np.ascontiguousarray(x[b].T).astype(np.float16) for b in range(B)]
    in_maps = []
    for c in range(NC):
        b, g = divmod(c, 4)
        gs = slice(g * G, (g + 1) * G)
        in_maps.append({
            "xT": xT[b],
            "wqT": np.ascontiguousarray(Wq[gs, :].T).astype(np.float16),
            "wkT": np.ascontiguousarray(Wk[gs, :].T).astype(np.float16),
            "wvT": np.ascontiguousarray(Wv[gs, :].T).astype(np.float16),
            "woT": np.ascontiguousarray(Wo[:, gs].T).astype(np.float16),
            "ropeC": C, "ropeS": Sm, "masks": masks, "ones": ones,
        })
    return in_maps


def _assemble(results):
    """results: list (per core) of {"out": [D, S] f32 partial outT}."""
    out = np.empty((B, S, D), np.float32)
    for b in range(B):
        acc = results[4 * b]["out"].astype(np.float32)
        for g in range(1, 4):
            acc = acc + results[4 * b + g]["out"]
        out[b] = acc.T
    return out


def kernel(x, token_positions, Wq, Wk, Wv, Wo):
    nc = _get_built()
    in_maps = _make_in_maps(x, token_positions, Wq, Wk, Wv, Wo)
    res = bass_utils.run_bass_kernel_spmd(
        nc, in_maps, core_ids=list(range(NC)), trace=False)
    return _assemble(res.results)


# revision 56
# speedup vs baseline: 1.0374x; 1.0374x over previous
"""Causal multi-head self-attention with RoPE on 8 TRN2 NeuronCores.

Problem: B=2, S=2048, D=2048, H=16 heads (dk=128), causal, interleaved RoPE.

Sharding (hardcoded): core c handles batch b = c // 4 and head group
g = c % 4 (heads 4g..4g+3, a 512-wide slice of d_model).  Attention is
embarrassingly parallel over (batch, head-group); the output projection is
row-parallel (each core contracts its 512-slice of attnout against the
matching 512 columns of Wo), so each core returns a full-size partial
output and the host sums the 4 partials per batch.

All device matmuls run in fp16 (full TensorE rate) with fp32 PSUM
accumulation.  Layout is fully transposed ("T" layout, feature dim on
partitions) so no on-device transposes are needed anywhere:

  xT [d, s] --(W.T @ .)--> QT/KT [dk, s] --RoPE--> scores.T [k, q]
  --exp--> P.T [k, q] --(V natural-layout matmul)--> OT [dv, q] --Wo--> outT

v2 scheduling: the softmax denominator no longer burns TensorE cycles as a
ones-stationary matmul per key block.  P tiles are accumulated on VectorE
(fp16) and a single ones-matmul per (head, q-chunk) does the final
partition reduction.  Because the exp on ScalarE ((n+352)/1.2 ns) is slower
than the remaining per-step PE work (QK+AV ~ n/1.2 ns), attention steps are
woven at fine grain with independent projection / output-projection matmuls
("filler") so the PE queue never head-of-line blocks on ScalarE.  Attention
for q-chunk sc runs inside iteration sc (right after its Q projection), so
no Scalar-paced attention epilogue remains; the kernel ends on the last
output-projection chain.  Input DMAs are issued from both the Sync and
GpSimd queues to halve the serial ~0.65us-per-descriptor issue cost at the
head of the kernel.
"""

import numpy as np

import concourse.bass as bass
import concourse.mybir as mybir
import concourse.tile as tile
from concourse import bacc
from concourse import bass_utils

B = 2
S = 2048
D = 2048
H = 16
DK = 128
HPC = 4          # heads per core
G = HPC * DK     # 512, d_model slice per core
NC = 8
THETA = 10000.0
SCALE = 1.0 / DK ** 0.5
EXP_BIAS = -5.0  # exp(s*SCALE - 5): keeps fp16 P in range; cancels in norm

FP16 = mybir.dt.float16
FP32 = mybir.dt.float32

_BUILT = None  # cached compiled Bass module


def _build_kernel(tc, out_d, xT_d, wqT_d, wkT_d, wvT_d, woT_d, ropeC_d,
                  ropeS_d, masks_d, ones_d):
    nc = tc.nc
    NSC = S // 512          # 4 s-chunks
    NDC = D // 128          # 16 d-chunks (contraction)
    shuffle_mask = [i + 1 if i % 2 == 0 else i - 1 for i in range(32)]

    with (
        tc.tile_pool(name="consts", bufs=1) as consts,
        tc.tile_pool(name="wqkv", bufs=1) as wqkv,
        tc.tile_pool(name="xin", bufs=2) as xin,
        tc.tile_pool(name="persist", bufs=1) as persist,
        tc.tile_pool(name="ropetmp", bufs=2) as ropetmp,
        tc.tile_pool(name="ptile", bufs=1) as ptile,
        tc.tile_pool(name="accp", bufs=4) as accp,
        tc.tile_pool(name="stage", bufs=2) as stage,
        tc.tile_pool(name="mm", bufs=3, space="PSUM") as mmp,
        tc.tile_pool(name="psST", bufs=2, space="PSUM") as psST,
        tc.tile_pool(name="psOT", bufs=2, space="PSUM") as psOT,
        tc.tile_pool(name="psZ", bufs=1, space="PSUM") as psZ,
    ):
        # weights in SBUF as [128, dc*512 + o]
        wq = wqkv.tile([128, NDC * G], FP16, tag="wq")
        wk = wqkv.tile([128, NDC * G], FP16, tag="wk")
        wv = wqkv.tile([128, NDC * G], FP16, tag="wv")
        wo = wqkv.tile([128, HPC * D], FP16, tag="wo")   # [128, hc*2048 + o]
        # persistent activations
        qrot = persist.tile([128, HPC * S], FP16, tag="qrot")  # [dk, h*S+s]
        krot = persist.tile([128, HPC * S], FP16, tag="krot")
        vN = persist.tile([128, (S // 128) * G], FP16, tag="vN")  # [s%, sb*G+dv]
        oT = persist.tile([128, HPC * S], FP16, tag="oT")      # [dv, h*S+q]

        ropeC = ropeS = maskT = onesT = expbias = None

        # PE warm-up: a short burst of dummy matmuls opens the HAM clock
        # gate (1.2 -> 2.4 GHz) while the first x/wq DMA pieces land; real
        # matmuls take over DMA-paced right after, keeping the PE active.
        warm = consts.tile([128, 512], FP16, tag="warm")
        nc.vector.memset(warm[:], 0.0)
        wps = psZ.tile([128, 512], FP32, tag="psZ", name="warmps")
        for _ in range(5):
            nc.tensor.matmul(wps[:], lhsT=warm[:, :128], rhs=warm[:],
                             start=True, stop=True)

        # ---- filler machinery: one closure == one PE matmul (plus any
        # cheap same-phase tail ops on other engines).  Attention steps
        # pump the cursor so the PE queue always holds independent work
        # while ScalarE chews on the exps.
        filler = []
        cursor = [0]
        credit = [0.0]

        def pump_upto(idx):
            while cursor[0] < min(idx, len(filler)):
                filler[cursor[0]]()
                cursor[0] += 1

        def pump(rate):
            credit[0] += rate
            n = int(credit[0])
            credit[0] -= n
            pump_upto(cursor[0] + n)

        def pump_all():
            pump_upto(len(filler))
            credit[0] = 0.0

        def emit_rope(dst, sc, h, ps):
            """RoPE from the PSUM projection result into dst[dk, h*S+sc*512]."""
            raw = ropetmp.tile([128, 512], FP16, tag="raw")
            nc.vector.tensor_copy(raw[:], ps[:])
            swp = ropetmp.tile([128, 512], FP16, tag="swp")
            nc.vector.stream_shuffle(swp[:], raw[:], shuffle_mask)
            t1 = ropetmp.tile([128, 512], FP16, tag="t1")
            csl = slice(sc * 512, (sc + 1) * 512)
            nc.vector.tensor_mul(t1[:], raw[:], ropeC[:, csl])
            t2 = ropetmp.tile([128, 512], FP16, tag="t2")
            nc.vector.tensor_mul(t2[:], swp[:], ropeS[:, csl])
            dsl = slice(h * S + sc * 512, h * S + (sc + 1) * 512)
            nc.vector.tensor_add(dst[:, dsl], t1[:], t2[:])

        def chain_qk(w_s, dst, xsc, sc, h, emit):
            """One head's QT/KT projection chain for chunk sc + fused RoPE."""
            ps = mmp.tile([128, 512], FP32, tag="mm", name="qkg")
            for dc in range(NDC):
                def mmfn(dc=dc, ps=ps):
                    nc.tensor.matmul(
                        ps[:],
                        lhsT=w_s[:, dc * G + h * 128: dc * G + (h + 1) * 128],
                        rhs=xsc[:, dc * 512:(dc + 1) * 512],
                        start=(dc == 0), stop=(dc == NDC - 1),
                        skip_group_check=True,
                    )
                    if dc == NDC - 1:
                        emit_rope(dst, sc, h, ps)
                emit(mmfn)

        def chains_qk_dcouter(w_s, dst, xsc, sc, heads=(0, 1, 2, 3)):
            """Heads' QT/KT chains with the dc loop OUTER, so each arriving
            2-dc DMA piece enables len(heads) matmuls per dc instead of 1.
            Only worth it in iteration 0 where the chains are DMA-paced;
            extra PSUM accumulators beyond 2 are borrowed from the (still
            idle) attention score pool."""
            pools = [mmp, mmp, psST, psST]
            tags = ["mm", "mm", "psST", "psST"]
            pss = [pools[j].tile([128, 512], FP32, tag=tags[j], name="qkg")
                   for j in range(len(heads))]
            for dc in range(NDC):
                for j, h in enumerate(heads):
                    nc.tensor.matmul(
                        pss[j][:],
                        lhsT=w_s[:, dc * G + h * 128: dc * G + (h + 1) * 128],
                        rhs=xsc[:, dc * 512:(dc + 1) * 512],
                        start=(dc == 0), stop=(dc == NDC - 1),
                        skip_group_check=True,
                    )
            for j, h in enumerate(heads):
                emit_rope(dst, sc, h, pss[j])

        def chain_v(xsc, sc, sb, emit):
            """V projection (natural layout) for row-block sb of chunk sc."""
            ps = mmp.tile([128, 512], FP32, tag="mm", name="vg")
            for dc in range(NDC):
                def mmfn(dc=dc, ps=ps):
                    nc.tensor.matmul(
                        ps[:],
                        lhsT=xsc[:, dc * 512 + sb * 128:
                                 dc * 512 + (sb + 1) * 128],
                        rhs=wv[:, dc * G:(dc + 1) * G],
                        start=(dc == 0), stop=(dc == NDC - 1),
                        skip_group_check=True,
                    )
                    if dc == NDC - 1:
                        sblk = sc * 4 + sb
                        nc.scalar.copy(vN[:, sblk * G:(sblk + 1) * G], ps[:])
                emit(mmfn)

        def chain_op(sc, ob, k, emit, split_q=False, fine_dma=False):
            """Output-projection chain: columns ob*128.. for s-chunk sc.
            split_q: issue half the output DMAs from the Scalar queue so the
            final drain isn't serialized on Sync (epilogue only).  fine_dma:
            additionally split this chain's output into two half-descriptors
            (for the last chains before kernel end)."""
            ps = mmp.tile([128, 512], FP32, tag="mm", name="psD")
            for hc in range(HPC):
                def mmfn(hc=hc, ps=ps):
                    nc.tensor.matmul(
                        ps[:],
                        lhsT=wo[:, hc * D + ob * 128: hc * D + (ob + 1) * 128],
                        rhs=oT[:, hc * S + sc * 512: hc * S + (sc + 1) * 512],
                        start=(hc == 0), stop=(hc == HPC - 1),
                        skip_group_check=True,
                    )
                    if hc == HPC - 1:
                        so = stage.tile([128, 512], FP16, tag="so", bufs=4)
                        if k % 2 == 0:
                            nc.vector.tensor_copy(so[:], ps[:])
                            dma_eng = nc.sync
                        else:
                            nc.scalar.copy(so[:], ps[:])
                            dma_eng = nc.scalar if split_q else nc.sync
                        orows = slice(ob * 128, (ob + 1) * 128)
                        if fine_dma:
                            # halves on two queues: per-descriptor DMA rate
                            # is ~1 engine's worth, so this halves the final
                            # output-drain latency after the last matmul.
                            for half, eng2 in ((0, nc.sync), (1, nc.scalar)):
                                csl = slice(sc * 512 + half * 256,
                                            sc * 512 + (half + 1) * 256)
                                eng2.dma_start(
                                    out=out_d[orows, csl],
                                    in_=so[:, half * 256:(half + 1) * 256],
                                )
                        else:
                            dma_eng.dma_start(
                                out=out_d[orows,
                                          sc * 512:(sc + 1) * 512],
                                in_=so[:],
                            )
                emit(mmfn)

        def attn_pair(hpair, qj, mark_k, mark_v):
            """Two heads' attention for q-chunk qj, woven with filler MMs.

            Front (QK matmul -> exp -> mask) runs LA steps ahead of the
            dependent AV matmul + VectorE P-accumulation; ~1.6 filler MMs
            per step cover ScalarE's (n+352)/1.2 exp cost so the PE queue
            never waits on the exp.  Diagonal blocks skip their fully-
            masked query-column prefix."""
            LA = 2
            ots = [psOT.tile([128, 512], FP32, tag="psOT", name=f"ot{i}")
                   for i in range(2)]
            accs = [accp.tile([128, 512], FP16, tag="acc", name=f"acc{i}")
                    for i in range(2)]
            nk = 4 * qj + 4
            steps = [(ki, i, h) for ki in range(nk)
                     for i, h in enumerate(hpair)]
            pending = []

            def emit_front(idx):
                ki, i, h = steps[idx]
                r = ki - 4 * qj
                qoff = 128 * r if r > 0 else 0  # fully-masked prefix width
                n = 512 - qoff
                qs0 = h * S + qj * 512
                if ki >= 4 * qj:  # diagonal: krot(qj, h) must be emitted
                    pump_upto(mark_k[h])
                st = psST.tile([128, 512], FP32, tag="psST")
                nc.tensor.matmul(
                    st[:, :n],
                    lhsT=krot[:, h * S + ki * 128: h * S + (ki + 1) * 128],
                    rhs=qrot[:, qs0 + qoff: qs0 + 512],
                    start=True, stop=True, skip_group_check=True,
                )
                pt = ptile.tile([128, 512], FP16, tag="pt", bufs=8)
                nc.scalar.activation(
                    pt[:, :n], st[:, :n],
                    mybir.ActivationFunctionType.Exp,
                    bias=expbias[:], scale=SCALE,
                )
                pa = pt
                if r >= 0:  # diagonal: zero the upper triangle
                    pm = ptile.tile([128, 512], FP16, tag="pm", bufs=5)
                    nc.vector.tensor_mul(
                        pm[:, :n], pt[:, :n],
                        maskT[:, r * 512 + qoff:(r + 1) * 512])
                    pa = pm
                return (ki, i, h, qoff, n, pa)

            def emit_back(item):
                ki, i, h, qoff, n, pa = item
                r = ki - 4 * qj
                if r >= 0:  # diagonal: vN(qj, sb=r) must be emitted
                    pump_upto(mark_v[r])
                nc.tensor.matmul(
                    ots[i][:, qoff:512],
                    lhsT=vN[:, ki * G + h * 128: ki * G + (h + 1) * 128],
                    rhs=pa[:, :n],
                    start=(ki == 0), stop=(ki == nk - 1),
                    skip_group_check=True,
                )
                if ki == 0:
                    nc.vector.tensor_copy(accs[i][:], pa[:])
                else:
                    nc.vector.tensor_add(accs[i][:, qoff:512],
                                         accs[i][:, qoff:512], pa[:, :n])

            for idx in range(len(steps)):
                pending.append(emit_front(idx))
                pump(0.8)
                if len(pending) > LA:
                    emit_back(pending.pop(0))
                    pump(0.8)
            for item in pending:
                emit_back(item)
                pump(0.8)
            # Cover the zt matmuls' cross-engine dependency (last exp ->
            # VectorE P-sum add) with independent matmuls so they don't
            # head-of-line block the PE queue at the pair boundary; when
            # the filler is dry (iteration 0's second pair) burn the
            # would-be idle on dummies, which also keeps the HAM activity
            # window busy so the clock gate stays at 2.4 GHz.
            pump(5)
            for _ in range(3):
                if cursor[0] >= len(filler):
                    nc.tensor.matmul(wps[:, :256], lhsT=warm[:, :128],
                                     rhs=warm[:, :256], start=True,
                                     stop=True, skip_group_check=True)
            for i, h in enumerate(hpair):
                zt = psZ.tile([128, 512], FP32, tag="psZ", name="zt")
                nc.tensor.matmul(zt[:], lhsT=onesT[:], rhs=accs[i][:],
                                 start=True, stop=True, skip_group_check=True)
                qsl = slice(h * S + qj * 512, h * S + (qj + 1) * 512)
                rz = stage.tile([128, 512], FP32, tag="rz")
                nc.vector.reciprocal_approx_fast(out=rz[:], in_=zt[:])
                nc.vector.tensor_mul(oT[:, qsl], ots[i][:], rz[:])

        run_now = lambda fn: fn()
        add_filler = filler.append

        # Pipeline: iteration sc does Q(sc) projection up front, then weaves
        # attention for q-chunk sc with the K(sc)/V(sc) chains and the
        # previous chunk's output projection as PE filler.
        for sc in range(NSC):
            assert cursor[0] == len(filler)
            filler.clear()
            cursor[0] = 0

            if sc == 0:
                xsc = xin.tile([128, NDC * 512], FP16, tag="xsc")
                # 1-dc pieces alternating between the Sync and Scalar issue
                # queues: first piece lands ~1us after the engines start, and
                # each piece unblocks 4 dc-outer matmuls.
                for dc in range(NDC):
                    eng = nc.sync if dc % 2 == 0 else nc.scalar
                    eng.dma_start(
                        out=xsc[:, dc * 512:(dc + 1) * 512],
                        in_=xT_d[dc * 128:(dc + 1) * 128, 0:512],
                    )
                for dc in range(0, NDC, 2):
                    nc.gpsimd.dma_start(
                        out=wq[:, dc * G:(dc + 2) * G]
                            .rearrange("p (c o) -> p c o", c=2),
                        in_=wqT_d[dc * 128:(dc + 2) * 128, :]
                            .rearrange("(c p) o -> p c o", p=128),
                    )
                for dc in range(0, NDC, 2):
                    nc.gpsimd.dma_start(
                        out=wk[:, dc * G:(dc + 2) * G]
                            .rearrange("p (c o) -> p c o", c=2),
                        in_=wkT_d[dc * 128:(dc + 2) * 128, :]
                            .rearrange("(c p) o -> p c o", p=128),
                    )
                ropeC = consts.tile_from(ropeC_d)    # [128, 2048] fp16
                ropeS = consts.tile_from(ropeS_d)
                maskT = consts.tile_from(masks_d)    # [128, 4*512] fp16
                expbias = consts.tile([128, 1], FP32, tag="expbias")
                nc.vector.memset(expbias[:], EXP_BIAS)
            else:
                xsc = xnext

            # Q projection for this chunk.  Iteration 0 is DMA-paced, so it
            # runs dc-outer (each arriving piece enables 8 matmuls); later
            # iterations have x resident and run per-head (PE-bound).
            if sc == 0:
                chains_qk_dcouter(wq, qrot, xsc, sc)
            else:
                for h in range(HPC):
                    chain_qk(wq, qrot, xsc, sc, h, run_now)

            if sc == 0:
                for dc in range(0, NDC, 2):
                    nc.gpsimd.dma_start(
                        out=wv[:, dc * G:(dc + 2) * G]
                            .rearrange("p (c o) -> p c o", c=2),
                        in_=wvT_d[dc * 128:(dc + 2) * 128, :]
                            .rearrange("(c p) o -> p c o", p=128),
                    )
                onesT = consts.tile_from(ones_d)     # [128, 128] fp16
                nc.gpsimd.dma_start(
                    out=wo[:].rearrange("p (c o) -> p c o", c=HPC),
                    in_=woT_d.rearrange("(c p) o -> p c o", p=128),
                )

            # Filler for the weave: K chains, V chains, out-proj(sc-1).
            mark_k = {}
            mark_v = {}
            if sc == 0:
                chains_qk_dcouter(wk, krot, xsc, sc)
                for h in range(HPC):
                    mark_k[h] = 0
            else:
                for h in range(HPC):
                    chain_qk(wk, krot, xsc, sc, h, add_filler)
                    mark_k[h] = len(filler)
            for sb in range(4):
                chain_v(xsc, sc, sb, add_filler)
                mark_v[sb] = len(filler)
            if sc >= 1:
                for ob in range(16):
                    chain_op(sc - 1, ob, ob, add_filler)

            attn_pair((0, 1), sc, mark_k, mark_v)
            # prefetch next chunk of x once iteration 0's critical input
            # DMAs have drained; it lands well before iteration sc+1.
            if sc + 1 < NSC:
                xnext = xin.tile([128, NDC * 512], FP16, tag="xsc")
                for dc in range(0, NDC, 2):
                    nc.sync.dma_start(
                        out=xnext[:, dc * 512:(dc + 2) * 512]
                            .rearrange("p (c s) -> p c s", c=2),
                        in_=xT_d[dc * 128:(dc + 2) * 128,
                                 (sc + 1) * 512:(sc + 2) * 512]
                            .rearrange("(c p) s -> p c s", p=128),
                    )
            attn_pair((2, 3), sc, mark_k, mark_v)
            pump_all()

        # epilogue: only the last chunk's output projection remains.  k=ob+1
        # so the final chain's stage copy lands on the faster Vector path.
        for ob in range(16):
            chain_op(NSC - 1, ob, ob + 1, run_now, split_q=True,
                     fine_dma=(ob >= 12))


def _get_built():
    global _BUILT
    if _BUILT is not None:
        return _BUILT
    nc = bacc.Bacc("TRN2", target_bir_lowering=False, debug=False,
                   enable_asserts=False, num_devices=NC)
    d = {}
    d["xT"] = nc.dram_tensor("xT", (D, S), FP16, kind="ExternalInput").ap()
    d["wqT"] = nc.dram_tensor("wqT", (D, G), FP16, kind="ExternalInput").ap()
    d["wkT"] = nc.dram_tensor("wkT", (D, G), FP16, kind="ExternalInput").ap()
    d["wvT"] = nc.dram_tensor("wvT", (D, G), FP16, kind="ExternalInput").ap()
    d["woT"] = nc.dram_tensor("woT", (G, D), FP16, kind="ExternalInput").ap()
    d["ropeC"] = nc.dram_tensor("ropeC", (DK, S), FP16,
                                kind="ExternalInput").ap()
    d["ropeS"] = nc.dram_tensor("ropeS", (DK, S), FP16,
                                kind="ExternalInput").ap()
    d["masks"] = nc.dram_tensor("masks", (DK, 4 * 512), FP16,
                                kind="ExternalInput").ap()
    d["ones"] = nc.dram_tensor("ones", (DK, DK), FP16,
                               kind="ExternalInput").ap()
    out_d = nc.dram_tensor("out", (D, S), FP16, kind="ExternalOutput").ap()
    with tile.TileContext(nc) as tc:
        _build_kernel(tc, out_d, d["xT"], d["wqT"], d["wkT"], d["wvT"],
                      d["woT"], d["ropeC"], d["ropeS"], d["masks"], d["ones"])
    nc.compile()
    _BUILT = nc
    return nc


def _host_tables(token_positions):
    pos = np.asarray(token_positions).astype(np.float64)       # [S]
    inv_freq = 1.0 / (THETA ** (np.arange(0, DK, 2, dtype=np.float64) / DK))
    ang = pos[None, :] * inv_freq[:, None]                     # [64, S]
    cos = np.cos(ang)
    sin = np.sin(ang)
    C = np.empty((DK, S), np.float16)
    Sm = np.empty((DK, S), np.float16)
    C[0::2] = cos
    C[1::2] = cos
    Sm[0::2] = -sin
    Sm[1::2] = sin
    # diagonal-block masks: mask_r[kr, qc] = 1 iff qc >= 128*r + kr
    masks = np.zeros((DK, 4 * 512), np.float16)
    kr = np.arange(128)[:, None]
    qc = np.arange(512)[None, :]
    for r in range(4):
        masks[:, r * 512:(r + 1) * 512] = (qc >= 128 * r + kr)
    ones = np.ones((DK, DK), np.float16)
    return C, Sm, masks, ones


def _make_in_maps(x, token_positions, Wq, Wk, Wv, Wo):
    C, Sm, masks, ones = _host_tables(token_positions)
    x = np.asarray(x, dtype=np.float32)
    Wq = np.asarray(Wq, dtype=np.float32)
    Wk = np.asarray(Wk, dtype=np.float32)
    Wv = np.asarray(Wv, dtype=np.float32)
    Wo = np.asarray(Wo, dtype=np.float32)
    xT = [np.ascontiguousarray(x[b].T).astype(np.float16) for b in range(B)]
    in_maps = []
    for c in range(NC):
        b, g = divmod(c, 4)
        gs = slice(g * G, (g + 1) * G)
        in_maps.append({
            "xT": xT[b],
            "wqT": np.ascontiguousarray(Wq[gs, :].T).astype(np.float16),
            "wkT": np.ascontiguousarray(Wk[gs, :].T).astype(np.float16),
            "wvT": np.ascontiguousarray(Wv[gs, :].T).astype(np.float16),
            "woT": np.ascontiguousarray(Wo[:, gs].T).astype(np.float16),
            "ropeC": C, "ropeS": Sm, "masks": masks, "ones": ones,
        })
    return in_maps


def _assemble(results):
    """results: list (per core) of {"out": [D, S] f32 partial outT}."""
    out = np.empty((B, S, D), np.float32)
    for b in range(B):
        acc = results[4 * b]["out"].astype(np.float32)
        for g in range(1, 4):
            acc = acc + results[4 * b + g]["out"]
        out[b] = acc.T
    return out


def kernel(x, token_positions, Wq, Wk, Wv, Wo):
    nc = _get_built()
    in_maps = _make_in_maps(x, token_positions, Wq, Wk, Wv, Wo)
    res = bass_utils.run_bass_kernel_spmd(
        nc, in_maps, core_ids=list(range(NC)), trace=False)
    return _assemble(res.results)


# revision 57
# speedup vs baseline: 1.0379x; 1.0005x over previous
"""Causal multi-head self-attention with RoPE on 8 TRN2 NeuronCores.

Problem: B=2, S=2048, D=2048, H=16 heads (dk=128), causal, interleaved RoPE.

Sharding (hardcoded): core c handles batch b = c // 4 and head group
g = c % 4 (heads 4g..4g+3, a 512-wide slice of d_model).  Attention is
embarrassingly parallel over (batch, head-group); the output projection is
row-parallel (each core contracts its 512-slice of attnout against the
matching 512 columns of Wo), so each core returns a full-size partial
output and the host sums the 4 partials per batch.

All device matmuls run in fp16 (full TensorE rate) with fp32 PSUM
accumulation.  Layout is fully transposed ("T" layout, feature dim on
partitions) so no on-device transposes are needed anywhere:

  xT [d, s] --(W.T @ .)--> QT/KT [dk, s] --RoPE--> scores.T [k, q]
  --exp--> P.T [k, q] --(V natural-layout matmul)--> OT [dv, q] --Wo--> outT

v2 scheduling: the softmax denominator no longer burns TensorE cycles as a
ones-stationary matmul per key block.  P tiles are accumulated on VectorE
(fp16) and a single ones-matmul per (head, q-chunk) does the final
partition reduction.  Because the exp on ScalarE ((n+352)/1.2 ns) is slower
than the remaining per-step PE work (QK+AV ~ n/1.2 ns), attention steps are
woven at fine grain with independent projection / output-projection matmuls
("filler") so the PE queue never head-of-line blocks on ScalarE.  Attention
for q-chunk sc runs inside iteration sc (right after its Q projection), so
no Scalar-paced attention epilogue remains; the kernel ends on the last
output-projection chain.  Input DMAs are issued from both the Sync and
GpSimd queues to halve the serial ~0.65us-per-descriptor issue cost at the
head of the kernel.
"""

import numpy as np

import concourse.bass as bass
import concourse.mybir as mybir
import concourse.tile as tile
from concourse import bacc
from concourse import bass_utils

B = 2
S = 2048
D = 2048
H = 16
DK = 128
HPC = 4          # heads per core
G = HPC * DK     # 512, d_model slice per core
NC = 8
THETA = 10000.0
SCALE = 1.0 / DK ** 0.5
EXP_BIAS = -5.0  # exp(s*SCALE - 5): keeps fp16 P in range; cancels in norm

FP16 = mybir.dt.float16
FP32 = mybir.dt.float32

_BUILT = None  # cached compiled Bass module


def _build_kernel(tc, out_d, xT_d, wqT_d, wkT_d, wvT_d, woT_d, ropeC_d,
                  ropeS_d, masks_d, ones_d):
    nc = tc.nc
    NSC = S // 512          # 4 s-chunks
    NDC = D // 128          # 16 d-chunks (contraction)
    shuffle_mask = [i + 1 if i % 2 == 0 else i - 1 for i in range(32)]

    with (
        tc.tile_pool(name="consts", bufs=1) as consts,
        tc.tile_pool(name="wqkv", bufs=1) as wqkv,
        tc.tile_pool(name="xin", bufs=2) as xin,
        tc.tile_pool(name="persist", bufs=1) as persist,
        tc.tile_pool(name="ropetmp", bufs=2) as ropetmp,
        tc.tile_pool(name="ptile", bufs=1) as ptile,
        tc.tile_pool(name="accp", bufs=4) as accp,
        tc.tile_pool(name="stage", bufs=2) as stage,
        tc.tile_pool(name="mm", bufs=3, space="PSUM") as mmp,
        tc.tile_pool(name="psST", bufs=2, space="PSUM") as psST,
        tc.tile_pool(name="psOT", bufs=2, space="PSUM") as psOT,
        tc.tile_pool(name="psZ", bufs=1, space="PSUM") as psZ,
    ):
        # weights in SBUF as [128, dc*512 + o]
        wq = wqkv.tile([128, NDC * G], FP16, tag="wq")
        wk = wqkv.tile([128, NDC * G], FP16, tag="wk")
        wv = wqkv.tile([128, NDC * G], FP16, tag="wv")
        wo = wqkv.tile([128, HPC * D], FP16, tag="wo")   # [128, hc*2048 + o]
        # persistent activations
        qrot = persist.tile([128, HPC * S], FP16, tag="qrot")  # [dk, h*S+s]
        krot = persist.tile([128, HPC * S], FP16, tag="krot")
        vN = persist.tile([128, (S // 128) * G], FP16, tag="vN")  # [s%, sb*G+dv]
        oT = persist.tile([128, HPC * S], FP16, tag="oT")      # [dv, h*S+q]

        ropeC = ropeS = maskT = onesT = expbias = None

        # PE warm-up: a short burst of dummy matmuls opens the HAM clock
        # gate (1.2 -> 2.4 GHz) while the first x/wq DMA pieces land; real
        # matmuls take over DMA-paced right after, keeping the PE active.
        warm = consts.tile([128, 512], FP16, tag="warm")
        nc.vector.memset(warm[:], 0.0)
        wps = psZ.tile([128, 512], FP32, tag="psZ", name="warmps")
        for _ in range(5):
            nc.tensor.matmul(wps[:], lhsT=warm[:, :128], rhs=warm[:],
                             start=True, stop=True)

        # ---- filler machinery: one closure == one PE matmul (plus any
        # cheap same-phase tail ops on other engines).  Attention steps
        # pump the cursor so the PE queue always holds independent work
        # while ScalarE chews on the exps.
        filler = []
        cursor = [0]
        credit = [0.0]

        def pump_upto(idx):
            while cursor[0] < min(idx, len(filler)):
                filler[cursor[0]]()
                cursor[0] += 1

        def pump(rate):
            credit[0] += rate
            n = int(credit[0])
            credit[0] -= n
            pump_upto(cursor[0] + n)

        def pump_all():
            pump_upto(len(filler))
            credit[0] = 0.0

        def emit_rope(dst, sc, h, ps):
            """RoPE from the PSUM projection result into dst[dk, h*S+sc*512]."""
            raw = ropetmp.tile([128, 512], FP16, tag="raw")
            nc.vector.tensor_copy(raw[:], ps[:])
            swp = ropetmp.tile([128, 512], FP16, tag="swp")
            nc.vector.stream_shuffle(swp[:], raw[:], shuffle_mask)
            t1 = ropetmp.tile([128, 512], FP16, tag="t1")
            csl = slice(sc * 512, (sc + 1) * 512)
            nc.vector.tensor_mul(t1[:], raw[:], ropeC[:, csl])
            t2 = ropetmp.tile([128, 512], FP16, tag="t2")
            nc.vector.tensor_mul(t2[:], swp[:], ropeS[:, csl])
            dsl = slice(h * S + sc * 512, h * S + (sc + 1) * 512)
            nc.vector.tensor_add(dst[:, dsl], t1[:], t2[:])

        def chain_qk(w_s, dst, xsc, sc, h, emit):
            """One head's QT/KT projection chain for chunk sc + fused RoPE."""
            ps = mmp.tile([128, 512], FP32, tag="mm", name="qkg")
            for dc in range(NDC):
                def mmfn(dc=dc, ps=ps):
                    nc.tensor.matmul(
                        ps[:],
                        lhsT=w_s[:, dc * G + h * 128: dc * G + (h + 1) * 128],
                        rhs=xsc[:, dc * 512:(dc + 1) * 512],
                        start=(dc == 0), stop=(dc == NDC - 1),
                        skip_group_check=True,
                    )
                    if dc == NDC - 1:
                        emit_rope(dst, sc, h, ps)
                emit(mmfn)

        def chains_qk_dcouter(w_s, dst, xsc, sc, heads=(0, 1, 2, 3)):
            """Heads' QT/KT chains with the dc loop OUTER, so each arriving
            2-dc DMA piece enables len(heads) matmuls per dc instead of 1.
            Only worth it in iteration 0 where the chains are DMA-paced;
            extra PSUM accumulators beyond 2 are borrowed from the (still
            idle) attention score pool."""
            pools = [mmp, mmp, psST, psST]
            tags = ["mm", "mm", "psST", "psST"]
            pss = [pools[j].tile([128, 512], FP32, tag=tags[j], name="qkg")
                   for j in range(len(heads))]
            for dc in range(NDC):
                for j, h in enumerate(heads):
                    nc.tensor.matmul(
                        pss[j][:],
                        lhsT=w_s[:, dc * G + h * 128: dc * G + (h + 1) * 128],
                        rhs=xsc[:, dc * 512:(dc + 1) * 512],
                        start=(dc == 0), stop=(dc == NDC - 1),
                        skip_group_check=True,
                    )
            for j, h in enumerate(heads):
                emit_rope(dst, sc, h, pss[j])

        def chain_v(xsc, sc, sb, emit):
            """V projection (natural layout) for row-block sb of chunk sc."""
            ps = mmp.tile([128, 512], FP32, tag="mm", name="vg")
            for dc in range(NDC):
                def mmfn(dc=dc, ps=ps):
                    nc.tensor.matmul(
                        ps[:],
                        lhsT=xsc[:, dc * 512 + sb * 128:
                                 dc * 512 + (sb + 1) * 128],
                        rhs=wv[:, dc * G:(dc + 1) * G],
                        start=(dc == 0), stop=(dc == NDC - 1),
                        skip_group_check=True,
                    )
                    if dc == NDC - 1:
                        sblk = sc * 4 + sb
                        nc.scalar.copy(vN[:, sblk * G:(sblk + 1) * G], ps[:])
                emit(mmfn)

        def chain_op(sc, ob, k, emit, split_q=False, fine_dma=False):
            """Output-projection chain: columns ob*128.. for s-chunk sc.
            split_q: issue half the output DMAs from the Scalar queue so the
            final drain isn't serialized on Sync (epilogue only).  fine_dma:
            additionally split this chain's output into two half-descriptors
            (for the last chains before kernel end)."""
            ps = mmp.tile([128, 512], FP32, tag="mm", name="psD")
            for hc in range(HPC):
                def mmfn(hc=hc, ps=ps):
                    nc.tensor.matmul(
                        ps[:],
                        lhsT=wo[:, hc * D + ob * 128: hc * D + (ob + 1) * 128],
                        rhs=oT[:, hc * S + sc * 512: hc * S + (sc + 1) * 512],
                        start=(hc == 0), stop=(hc == HPC - 1),
                        skip_group_check=True,
                    )
                    if hc == HPC - 1:
                        so = stage.tile([128, 512], FP16, tag="so", bufs=4)
                        if k % 2 == 0:
                            nc.vector.tensor_copy(so[:], ps[:])
                            dma_eng = nc.sync
                        else:
                            nc.scalar.copy(so[:], ps[:])
                            dma_eng = nc.scalar if split_q else nc.sync
                        orows = slice(ob * 128, (ob + 1) * 128)
                        if fine_dma:
                            # halves on two queues: per-descriptor DMA rate
                            # is ~1 engine's worth, so this halves the final
                            # output-drain latency after the last matmul.
                            for half, eng2 in ((0, nc.sync), (1, nc.scalar)):
                                csl = slice(sc * 512 + half * 256,
                                            sc * 512 + (half + 1) * 256)
                                eng2.dma_start(
                                    out=out_d[orows, csl],
                                    in_=so[:, half * 256:(half + 1) * 256],
                                )
                        else:
                            dma_eng.dma_start(
                                out=out_d[orows,
                                          sc * 512:(sc + 1) * 512],
                                in_=so[:],
                            )
                emit(mmfn)

        def attn_pair(hpair, qj, mark_k, mark_v):
            """Two heads' attention for q-chunk qj, woven with filler MMs.

            Front (QK matmul -> exp -> mask) runs LA steps ahead of the
            dependent AV matmul + VectorE P-accumulation; ~1.6 filler MMs
            per step cover ScalarE's (n+352)/1.2 exp cost so the PE queue
            never waits on the exp.  Diagonal blocks skip their fully-
            masked query-column prefix."""
            LA = 2
            ots = [psOT.tile([128, 512], FP32, tag="psOT", name=f"ot{i}")
                   for i in range(2)]
            accs = [accp.tile([128, 512], FP16, tag="acc", name=f"acc{i}")
                    for i in range(2)]
            nk = 4 * qj + 4
            steps = [(ki, i, h) for ki in range(nk)
                     for i, h in enumerate(hpair)]
            pending = []

            def emit_front(idx):
                ki, i, h = steps[idx]
                r = ki - 4 * qj
                qoff = 128 * r if r > 0 else 0  # fully-masked prefix width
                n = 512 - qoff
                qs0 = h * S + qj * 512
                if ki >= 4 * qj:  # diagonal: krot(qj, h) must be emitted
                    pump_upto(mark_k[h])
                st = psST.tile([128, 512], FP32, tag="psST")
                nc.tensor.matmul(
                    st[:, :n],
                    lhsT=krot[:, h * S + ki * 128: h * S + (ki + 1) * 128],
                    rhs=qrot[:, qs0 + qoff: qs0 + 512],
                    start=True, stop=True, skip_group_check=True,
                )
                pt = ptile.tile([128, 512], FP16, tag="pt", bufs=8)
                nc.scalar.activation(
                    pt[:, :n], st[:, :n],
                    mybir.ActivationFunctionType.Exp,
                    bias=expbias[:], scale=SCALE,
                )
                pa = pt
                if r >= 0:  # diagonal: zero the upper triangle
                    pm = ptile.tile([128, 512], FP16, tag="pm", bufs=5)
                    nc.vector.tensor_mul(
                        pm[:, :n], pt[:, :n],
                        maskT[:, r * 512 + qoff:(r + 1) * 512])
                    pa = pm
                return (ki, i, h, qoff, n, pa)

            def emit_back(item):
                ki, i, h, qoff, n, pa = item
                r = ki - 4 * qj
                if r >= 0:  # diagonal: vN(qj, sb=r) must be emitted
                    pump_upto(mark_v[r])
                nc.tensor.matmul(
                    ots[i][:, qoff:512],
                    lhsT=vN[:, ki * G + h * 128: ki * G + (h + 1) * 128],
                    rhs=pa[:, :n],
                    start=(ki == 0), stop=(ki == nk - 1),
                    skip_group_check=True,
                )
                if ki == 0:
                    nc.vector.tensor_copy(accs[i][:], pa[:])
                else:
                    nc.vector.tensor_add(accs[i][:, qoff:512],
                                         accs[i][:, qoff:512], pa[:, :n])

            for idx in range(len(steps)):
                pending.append(emit_front(idx))
                pump(0.8)
                if len(pending) > LA:
                    emit_back(pending.pop(0))
                    pump(0.8)
            for item in pending:
                emit_back(item)
                pump(0.8)
            # Cover the zt matmuls' cross-engine dependency (last exp ->
            # VectorE P-sum add) with independent matmuls so they don't
            # head-of-line block the PE queue at the pair boundary; when
            # the filler is dry (iteration 0's second pair) burn the
            # would-be idle on dummies, which also keeps the HAM activity
            # window busy so the clock gate stays at 2.4 GHz.
            pump(5)
            for _ in range(3):
                if cursor[0] >= len(filler):
                    nc.tensor.matmul(wps[:, :256], lhsT=warm[:, :128],
                                     rhs=warm[:, :256], start=True,
                                     stop=True, skip_group_check=True)
            for i, h in enumerate(hpair):
                zt = psZ.tile([128, 512], FP32, tag="psZ", name="zt")
                nc.tensor.matmul(zt[:], lhsT=onesT[:], rhs=accs[i][:],
                                 start=True, stop=True, skip_group_check=True)
                qsl = slice(h * S + qj * 512, h * S + (qj + 1) * 512)
                rz = stage.tile([128, 512], FP32, tag="rz")
                nc.vector.reciprocal_approx_fast(out=rz[:], in_=zt[:])
                nc.vector.tensor_mul(oT[:, qsl], ots[i][:], rz[:])

        run_now = lambda fn: fn()
        add_filler = filler.append

        # Pipeline: iteration sc does Q(sc) projection up front, then weaves
        # attention for q-chunk sc with the K(sc)/V(sc) chains and the
        # previous chunk's output projection as PE filler.
        for sc in range(NSC):
            assert cursor[0] == len(filler)
            filler.clear()
            cursor[0] = 0

            if sc == 0:
                xsc = xin.tile([128, NDC * 512], FP16, tag="xsc")
                # The very first matmuls gate on x-dc0 and wq-dc0; a single
                # descriptor only gets ~1 DMA engine's bandwidth (20-60GB/s,
                # assignment luck), so those two pieces go as 64KB halves on
                # different queues to halve their worst-case latency.  Later
                # dcs stay as 1-dc pieces alternating Sync/Scalar so each
                # piece still unblocks 4 dc-outer matmuls.
                for half in range(2):
                    hs = slice(half * 256, (half + 1) * 256)
                    [nc.sync, nc.scalar][half].dma_start(
                        out=xsc[:, half * 256:(half + 1) * 256],
                        in_=xT_d[0:128, hs],
                    )
                    [nc.gpsimd, nc.sync][half].dma_start(
                        out=wq[:, half * 256:(half + 1) * 256],
                        in_=wqT_d[0:128, hs],
                    )
                for dc in range(1, NDC):
                    eng = nc.sync if dc % 2 == 0 else nc.scalar
                    eng.dma_start(
                        out=xsc[:, dc * 512:(dc + 1) * 512],
                        in_=xT_d[dc * 128:(dc + 1) * 128, 0:512],
                    )
                nc.gpsimd.dma_start(
                    out=wq[:, G:2 * G],
                    in_=wqT_d[128:256, :],
                )
                for dc in range(2, NDC, 2):
                    nc.gpsimd.dma_start(
                        out=wq[:, dc * G:(dc + 2) * G]
                            .rearrange("p (c o) -> p c o", c=2),
                        in_=wqT_d[dc * 128:(dc + 2) * 128, :]
                            .rearrange("(c p) o -> p c o", p=128),
                    )
                for dc in range(0, NDC, 2):
                    nc.gpsimd.dma_start(
                        out=wk[:, dc * G:(dc + 2) * G]
                            .rearrange("p (c o) -> p c o", c=2),
                        in_=wkT_d[dc * 128:(dc + 2) * 128, :]
                            .rearrange("(c p) o -> p c o", p=128),
                    )
                ropeC = consts.tile_from(ropeC_d)    # [128, 2048] fp16
                ropeS = consts.tile_from(ropeS_d)
                maskT = consts.tile_from(masks_d)    # [128, 4*512] fp16
                expbias = consts.tile([128, 1], FP32, tag="expbias")
                nc.vector.memset(expbias[:], EXP_BIAS)
            else:
                xsc = xnext

            # Q projection for this chunk.  Iteration 0 is DMA-paced, so it
            # runs dc-outer (each arriving piece enables 8 matmuls); later
            # iterations have x resident and run per-head (PE-bound).
            if sc == 0:
                chains_qk_dcouter(wq, qrot, xsc, sc)
            else:
                for h in range(HPC):
                    chain_qk(wq, qrot, xsc, sc, h, run_now)

            if sc == 0:
                for dc in range(0, NDC, 2):
                    nc.gpsimd.dma_start(
                        out=wv[:, dc * G:(dc + 2) * G]
                            .rearrange("p (c o) -> p c o", c=2),
                        in_=wvT_d[dc * 128:(dc + 2) * 128, :]
                            .rearrange("(c p) o -> p c o", p=128),
                    )
                onesT = consts.tile_from(ones_d)     # [128, 128] fp16
                nc.gpsimd.dma_start(
                    out=wo[:].rearrange("p (c o) -> p c o", c=HPC),
                    in_=woT_d.rearrange("(c p) o -> p c o", p=128),
                )

            # Filler for the weave: K chains, V chains, out-proj(sc-1).
            mark_k = {}
            mark_v = {}
            if sc == 0:
                chains_qk_dcouter(wk, krot, xsc, sc)
                for h in range(HPC):
                    mark_k[h] = 0
            else:
                for h in range(HPC):
                    chain_qk(wk, krot, xsc, sc, h, add_filler)
                    mark_k[h] = len(filler)
            for sb in range(4):
                chain_v(xsc, sc, sb, add_filler)
                mark_v[sb] = len(filler)
            if sc >= 1:
                for ob in range(16):
                    chain_op(sc - 1, ob, ob, add_filler)

            attn_pair((0, 1), sc, mark_k, mark_v)
            # prefetch next chunk of x once iteration 0's critical input
            # DMAs have drained; it lands well before iteration sc+1.
            if sc + 1 < NSC:
                xnext = xin.tile([128, NDC * 512], FP16, tag="xsc")
                for dc in range(0, NDC, 2):
                    nc.sync.dma_start(
                        out=xnext[:, dc * 512:(dc + 2) * 512]
                            .rearrange("p (c s) -> p c s", c=2),
                        in_=xT_d[dc * 128:(dc + 2) * 128,
                                 (sc + 1) * 512:(sc + 2) * 512]
                            .rearrange("(c p) s -> p c s", p=128),
                    )
            attn_pair((2, 3), sc, mark_k, mark_v)
            pump_all()

        # epilogue: only the last chunk's output projection remains.  k=ob+1
        # so the final chain's stage copy lands on the faster Vector path.
        for ob in range(16):
            chain_op(NSC - 1, ob, ob + 1, run_now, split_q=True,
                     fine_dma=(ob >= 12))


def _get_built():
    global _BUILT
    if _BUILT is not None:
        return _BUILT
    nc = bacc.Bacc("TRN2", target_bir_lowering=False, debug=False,
                   enable_asserts=False, num_devices=NC)
    d = {}
    d["xT"] = nc.dram_tensor("xT", (D, S), FP16, kind="ExternalInput").ap()
    d["wqT"] = nc.dram_tensor("wqT", (D, G), FP16, kind="ExternalInput").ap()
    d["wkT"] = nc.dram_tensor("wkT", (D, G), FP16, kind="ExternalInput").ap()
    d["wvT"] = nc.dram_tensor("wvT", (D, G), FP16, kind="ExternalInput").ap()
    d["woT"] = nc.dram_tensor("woT", (G, D), FP16, kind="ExternalInput").ap()
    d["ropeC"] = nc.dram_tensor("ropeC", (DK, S), FP16,
                                kind="ExternalInput").ap()
    d["ropeS"] = nc.dram_tensor("ropeS", (DK, S), FP16,
                                kind="ExternalInput").ap()
    d["masks"] = nc.dram_tensor("masks", (DK, 4 * 512), FP16,
                                kind="ExternalInput").ap()
    d["ones"] = nc.dram_tensor("ones", (DK, DK), FP16,
                               kind="ExternalInput").ap()
    out_d = nc.dram_tensor("out", (D, S), FP16, kind="ExternalOutput").ap()
    with tile.TileContext(nc) as tc:
        _build_kernel(tc, out_d, d["xT"], d["wqT"], d["wkT"], d["wvT"],
                      d["woT"], d["ropeC"], d["ropeS"], d["masks"], d["ones"])
    nc.compile()
    _BUILT = nc
    return nc


def _host_tables(token_positions):
    pos = np.asarray(token_positions).astype(np.float64)       # [S]
    inv_freq = 1.0 / (THETA ** (np.arange(0, DK, 2, dtype=np.float64) / DK))
    ang = pos[None, :] * inv_freq[:, None]                     # [64, S]
    cos = np.cos(ang)
    sin = np.sin(ang)
    C = np.empty((DK, S), np.float16)
    Sm = np.empty((DK, S), np.float16)
    C[0::2] = cos
    C[1::2] = cos
    Sm[0::2] = -sin
    Sm[1::2] = sin
    # diagonal-block masks: mask_r[kr, qc] = 1 iff qc >= 128*r + kr
    masks = np.zeros((DK, 4 * 512), np.float16)
    kr = np.arange(128)[:, None]
    qc = np.arange(512)[None, :]
    for r in range(4):
        masks[:, r * 512:(r + 1) * 512] = (qc >= 128 * r + kr)
    ones = np.ones((DK, DK), np.float16)
    return C, Sm, masks, ones


def _make_in_maps(x, token_positions, Wq, Wk, Wv, Wo):
    C, Sm, masks, ones = _host_tables(token_positions)
    x = np.asarray(x, dtype=np.float32)
    Wq = np.asarray(Wq, dtype=np.float32)
    Wk = np.asarray(Wk, dtype=np.float32)
    Wv = np.asarray(Wv, dtype=np.float32)
    Wo = np.asarray(Wo, dtype=np.float32)
    xT = [np.ascontiguousarray(x[b].T).astype(np.float16) for b in range(B)]
    in_maps = []
    for c in range(NC):
        b, g = divmod(c, 4)
        gs = slice(g * G, (g + 1) * G)
        in_maps.append({
            "xT": xT[b],
            "wqT": np.ascontiguousarray(Wq[gs, :].T).astype(np.float16),
            "wkT": np.ascontiguousarray(Wk[gs, :].T).astype(np.float16),
            "wvT": np.ascontiguousarray(Wv[gs, :].T).astype(np.float16),
            "woT": np.ascontiguousarray(Wo[:, gs].T).astype(np.float16),
            "ropeC": C, "ropeS": Sm, "masks": masks, "ones": ones,
        })
    return in_maps


def _assemble(results):
    """results: list (per core) of {"out": [D, S] f32 partial outT}."""
    out = np.empty((B, S, D), np.float32)
    for b in range(B):
        acc = results[4 * b]["out"].astype(np.float32)
        for g in range(1, 4):
            acc = acc + results[4 * b + g]["out"]
        out[b] = acc.T
    return out


def kernel(x, token_positions, Wq, Wk, Wv, Wo):
    nc = _get_built()
    in_maps = _make_in_maps(x, token_positions, Wq, Wk, Wv, Wo)
    res = bass_utils.run_bass_kernel_spmd(
        nc, in_maps, core_ids=list(range(NC)), trace=False)
    return _assemble(res.results)
